# revision 2
# baseline (speedup 1.0000x reference)
"""RWKV WKV recurrence kernel for Trainium2 (8 NeuronCores) — v3.

Problem: B=8, T=2048, H=768 fp32.
  u = time_first; w = -exp(time_decay); d = exp(w)
  A_t = d*A_{t-1} + e^{k_t} v_t ;  B_t = d*B_{t-1} + e^{k_t}
  wkv_t = (A_{t-1} + eu*e^{k_t} v_t) / (B_{t-1} + eu*e^{k_t})

Mapping: data-parallel over batch (1 batch per core); host pre-transposes
k/v to [H, T] bf16 in PHASE-MAJOR time layout (nph=4 planes) and
transposes the output back. Per core, 6 h-blocks of 128 channels.

v3 engine assignment (balanced after per-op HW measurement; v2 was
DVE-bound at 82% busy):
  Scalar : exp(k+u) -> eku ; d*ev / d^2*X1ev scale-multiplies
           (activation Copy with per-partition scale column — lives in
           the same act table as Exp, so zero table reloads) ; num
           PSUM->SBUF bf16 drain.
  GpSimd : ekv = eku*v (Pool is SBUF-only; TT mult is its one fast op).
  DVE    : combine adds X1 = (d*ev) + od, X2 = (d^2*X1ev) + X1od
           (bf16 2x tensor_tensor), the two per-block scans,
           reciprocal_approx_fast (PSUM fp32 -> bf16), and a share of
           the final wkv = num*rden multiplies.
  PE     : phase downsweep — per tensor 12 matmuls of [P,P]@[P,512]
           (s'-terms diag(reu*d^j), P_c terms, identity-z) accumulated
           into PSUM num/den chunks.
The T recurrence is the same phase decomposition as v2: 4 interleaved
phases pair-combined twice, ONE scan of length 512 with decay d^4, and
a TensorE down-sweep reconstructing all phase planes of A_{t-1}.
"""

import numpy as np
from contextlib import ExitStack

import concourse.bass as bass
import concourse.tile as tile
from concourse import mybir, bacc
from concourse.bass_utils import run_bass_kernel_spmd
from concourse.masks import make_identity
from concourse.dve_ops import RECIP_APPROX_FAST_CONSTS, RECIPROCAL_APPROX_FAST

import os

B, T, H = 8, 2048, 768
P = 128
NHB = H // P    # 6 h-blocks
NPH = 4
S = T // NPH    # 512
L = 2
NPOW = L + 1

F32 = mybir.dt.float32
BF16 = mybir.dt.bfloat16

# tuning knobs
POOL_MUL = int(os.environ.get("WKV_POOL_MUL", "6"))    # of 12 chunk-muls on Pool
EKV_DVE = int(os.environ.get("WKV_EKV_DVE", "0"))      # blocks with ekv on DVE
RECIP_F32 = os.environ.get("WKV_RECIP_F32", "0") == "1"
X2F32 = os.environ.get("WKV_X2F32", "0") == "1"
SCALE_SC = os.environ.get("WKV_SCALE_SC", "1") == "1"  # l1/l2 scale on Scalar

_cache = {}


def _build(reps=1, hw_loop=False, nph=NPH, **unused):
    assert nph == NPH
    nc = bacc.Bacc()
    k_in = nc.dram_tensor("k", [H, T], BF16, kind="ExternalInput")
    v_in = nc.dram_tensor("v", [H, T], BF16, kind="ExternalInput")
    dp_in = nc.dram_tensor("dp", [NPOW, H], F32, kind="ExternalInput")
    u_in = nc.dram_tensor("u", [H], F32, kind="ExternalInput")
    reu_in = nc.dram_tensor("reu", [H], F32, kind="ExternalInput")
    rud_in = nc.dram_tensor("rud", [NPH, H], F32, kind="ExternalInput")
    o = nc.dram_tensor("o", [H, T], BF16, kind="ExternalOutput")

    mult, add = mybir.AluOpType.mult, mybir.AluOpType.add
    Copy = mybir.ActivationFunctionType.Copy
    Exp = mybir.ActivationFunctionType.Exp

    with tile.TileContext(nc) as tc, ExitStack() as ctx:
        consts = ctx.enter_context(tc.tile_pool(name="consts", bufs=1))
        work = ctx.enter_context(tc.tile_pool(
            name="work", bufs=int(os.environ.get("WKV_BUFS", "2"))))
        psum = ctx.enter_context(tc.tile_pool(name="psum", bufs=2, space="PSUM"))

        dp_cols = consts.tile([P, NPOW * NHB], F32)
        u_cols = consts.tile([P, NHB], F32)
        nc.sync.dma_start(
            out=dp_cols, in_=dp_in.rearrange("n (f p) -> p (n f)", p=P))
        nc.sync.dma_start(out=u_cols, in_=u_in.rearrange("(f p) -> p f", p=P))

        ident16 = consts.tile([P, P], BF16)
        make_identity(nc, ident16[:])
        rud_cols = consts.tile([P, NPH * NHB], F32)
        nc.sync.dma_start(
            out=rud_cols, in_=rud_in.rearrange("n (f p) -> p (n f)", p=P))
        diag_rud = consts.tile([P, NPH * NHB * P], BF16)
        for _j in range(NPH):
            for _hb in range(NHB):
                _o = (_j * NHB + _hb) * P
                nc.vector.tensor_scalar_mul(
                    out=diag_rud[:, _o:_o + P], in0=ident16,
                    scalar1=rud_cols[:, _j * NHB + _hb:_j * NHB + _hb + 1])

        def dgj(j, hb):
            _o = (j * NHB + hb) * P
            return diag_rud[:, _o:_o + P]

        def dpcol(l, hb):
            return dp_cols[:, l * NHB + hb:l * NHB + hb + 1]

        mul_sched = [True] * POOL_MUL + [False] * (12 - POOL_MUL)

        import contextlib
        loop_ctx = tc.For_i(0, reps) if hw_loop else contextlib.nullcontext()
        with loop_ctx:
          for rep in range(1 if hw_loop else reps):
            for hb in range(NHB):
                ucol = u_cols[:, hb:hb + 1]
                hs = slice(hb * P, (hb + 1) * P)

                kb = work.tile([P, T], BF16, tag="kb")
                nc.sync.dma_start(out=kb, in_=k_in[hs, :])
                vb = work.tile([P, T], BF16, tag="vb")
                nc.sync.dma_start(out=vb, in_=v_in[hs, :])

                eku = work.tile([P, T], BF16, tag="eku")
                nc.scalar.activation(out=eku, in_=kb, func=Exp, bias=ucol)
                ekv = work.tile([P, T], BF16, tag="ekv")
                if hb < EKV_DVE:
                    nc.vector.tensor_mul(out=ekv, in0=eku, in1=vb)
                else:
                    nc.gpsimd.tensor_mul(out=ekv, in0=eku, in1=vb)

                sps, X1s, zs = [], [], []
                x2dt = F32 if X2F32 else BF16
                for (z, pfx) in ((ekv, "a"), (eku, "b")):
                    pair = z[:, 0:T].rearrange("p (a s) -> p a s", s=2 * S)
                    ev = pair[:, :, 0:S]
                    od = pair[:, :, S:2 * S]
                    # l1: X1 = d*ev + od
                    X1 = work.tile([P, 2 * S], BF16, tag=f"{pfx}X1")
                    X1v = X1.rearrange("p (a s) -> p a s", s=S)
                    if SCALE_SC:
                        T1 = work.tile([P, 2 * S], BF16, tag=f"{pfx}T1")
                        T1v = T1.rearrange("p (a s) -> p a s", s=S)
                        nc.scalar.activation(out=T1v, in_=ev, func=Copy,
                                             scale=dpcol(0, hb))
                        nc.vector.tensor_tensor(out=X1v, in0=T1v, in1=od,
                                                op=add)
                    else:
                        nc.vector.scalar_tensor_tensor(
                            out=X1v, in0=ev, scalar=dpcol(0, hb), in1=od,
                            op0=mult, op1=add)
                    # l2: X2 = d^2*X1_0 + X1_1
                    X2 = work.tile([P, S], x2dt, tag=f"{pfx}X2")
                    if SCALE_SC:
                        T2 = work.tile([P, S], BF16, tag=f"{pfx}T2")
                        nc.scalar.activation(out=T2, in_=X1[:, 0:S], func=Copy,
                                             scale=dpcol(1, hb))
                        nc.vector.tensor_tensor(out=X2, in0=T2,
                                                in1=X1[:, S:2 * S], op=add)
                    else:
                        nc.vector.scalar_tensor_tensor(
                            out=X2, in0=X1[:, 0:S], scalar=dpcol(1, hb),
                            in1=X1[:, S:2 * S], op0=mult, op1=add)
                    # scan with decay d^4; s[0]=0 so sp[sig] = A(4sig-1)
                    s = work.tile([P, S + 1], BF16, tag=f"{pfx}s")
                    nc.gpsimd.memset(s[:, 0:1], 0.0)
                    nc.vector.tensor_tensor_scan(
                        out=s[:, 1:S + 1],
                        data0=dpcol(2, hb).broadcast_to([P, S]),
                        data1=X2, initial=0.0, op0=mult, op1=add)
                    sps.append(s[:, 0:S])
                    X1s.append(X1)
                    zs.append(z)

                # A_{t-1} planes: p0: s'; p1: d s'+z0; p2: d^2 s'+X1_0;
                # p3: d^3 s'+d X1_0+z2   (all scaled by reu via diag_rud)
                def terms_for(spx, X1x, zx, c):
                    z0 = zx[:, 0:S]
                    z2 = zx[:, 2 * S:3 * S]
                    x10 = X1x[:, 0:S]
                    tl = [
                        [(dgj(0, hb), spx)],
                        [(dgj(1, hb), spx), (dgj(0, hb), z0)],
                        [(dgj(2, hb), spx), (dgj(0, hb), x10)],
                        [(dgj(3, hb), spx), (dgj(1, hb), x10),
                         (dgj(0, hb), z2)],
                    ][c]
                    return tl + [(ident16, zx[:, c * S:(c + 1) * S])]

                for ch in range(2):
                    num_h = psum.tile([P, 2 * S], F32, tag="numh")
                    den_h = psum.tile([P, 2 * S], F32, tag="denh")
                    for (acc, spx, X1x, zx) in (
                            (num_h, sps[0], X1s[0], zs[0]),
                            (den_h, sps[1], X1s[1], zs[1])):
                        for cc in range(2):
                            c = ch * 2 + cc
                            tl = terms_for(spx, X1x, zx, c)
                            for ti, (dgm, mv) in enumerate(tl):
                                nc.tensor.matmul(
                                    out=acc[:, cc * S:(cc + 1) * S],
                                    lhsT=dgm, rhs=mv, start=(ti == 0),
                                    stop=(ti == len(tl) - 1))
                    numb = work.tile([P, 2 * S], BF16, tag="numb")
                    nc.scalar.activation(out=numb, in_=num_h[:], func=Copy)
                    rdt = F32 if RECIP_F32 else BF16
                    rdenb = work.tile([P, 2 * S], rdt, tag="rdenb")
                    c = RECIP_APPROX_FAST_CONSTS
                    nc.vector._custom_dve(
                        RECIPROCAL_APPROX_FAST, out=rdenb[:], in0=den_h[:],
                        s0=c["s0"], s1=c["s1"], imm2=c["imm2"])
                    wkv_h = work.tile([P, 2 * S], BF16, tag="wkvh")
                    if mul_sched[hb * 2 + ch]:
                        nc.gpsimd.tensor_mul(out=wkv_h, in0=numb, in1=rdenb)
                    else:
                        nc.vector.tensor_mul(out=wkv_h, in0=numb, in1=rdenb)
                    nc.sync.dma_start(
                        out=o[hs, ch * 2 * S:(ch + 1) * 2 * S], in_=wkv_h)

    nc.finalize()
    return nc


def prep_host_inputs(key, value, time_decay, time_first, nph=NPH):
    """Host-side prep: [B,T,H] f32 -> per-core [H,T] bf16 phase-major."""
    bf16 = mybir.dt.np(BF16)

    def to_planes(x):
        xt = np.ascontiguousarray(x.T)                  # [H, T]
        xp = xt.reshape(H, S, nph).transpose(0, 2, 1)   # [H, nph, S]
        return np.ascontiguousarray(xp.reshape(H, T)).astype(bf16)

    td64 = np.asarray(time_decay, np.float64)
    u64 = np.asarray(time_first, np.float64)
    d = np.exp(-np.exp(td64))
    dp = np.stack([(d ** (1 << l)) for l in range(L + 1)], axis=0)
    dp = dp.astype(np.float32)
    u = u64.astype(np.float32)
    reu64 = np.exp(-u64)
    reu = reu64.astype(np.float32)
    base = {
        "dp": dp, "u": u, "reu": reu,
        "rud": np.stack([reu64 * (d ** j) for j in range(nph)],
                        axis=0).astype(np.float32),
    }
    return [
        {"k": to_planes(key[b]), "v": to_planes(value[b]), **base}
        for b in range(B)
    ]


def unprep_host_output(o_planes, nph=NPH):
    """[H, T] bf16 phase-major -> [T, H] f32."""
    x = o_planes.astype(np.float32).reshape(H, nph, S)
    xt = x.transpose(0, 2, 1).reshape(H, T)  # [H, T] time-major
    return np.ascontiguousarray(xt.T)


def kernel(key, value, time_decay, time_first):
    key = np.ascontiguousarray(key, dtype=np.float32)
    value = np.ascontiguousarray(value, dtype=np.float32)
    in_maps = prep_host_inputs(key, value, time_decay, time_first)

    if "nc" not in _cache:
        _cache["nc"] = _build(reps=1)
    nc = _cache["nc"]

    res = run_bass_kernel_spmd(nc, in_maps, core_ids=list(range(B)))
    out = np.stack([unprep_host_output(r["o"]) for r in res.results], axis=0)
    return np.ascontiguousarray(out)


if __name__ == "__main__":
    rng = np.random.default_rng(0)
    ktest = rng.standard_normal((B, T, H), dtype=np.float32)
    vtest = rng.standard_normal((B, T, H), dtype=np.float32)
    td = rng.standard_normal(H).astype(np.float32)
    tf = rng.standard_normal(H).astype(np.float32)
    out = kernel(ktest, vtest, td, tf)
    print("out", out.shape, out.dtype, np.abs(out).max())


# revision 4
# speedup vs baseline: 1.0193x; 1.0193x over previous
"""RWKV WKV recurrence kernel for Trainium2 (8 NeuronCores) — v3.

Problem: B=8, T=2048, H=768 fp32.
  u = time_first; w = -exp(time_decay); d = exp(w)
  A_t = d*A_{t-1} + e^{k_t} v_t ;  B_t = d*B_{t-1} + e^{k_t}
  wkv_t = (A_{t-1} + eu*e^{k_t} v_t) / (B_{t-1} + eu*e^{k_t})

Mapping: data-parallel over batch (1 batch per core); host pre-transposes
k/v to [H, T] bf16 in PHASE-MAJOR time layout (nph=4 planes) and
transposes the output back. Per core, 6 h-blocks of 128 channels.

v3 engine assignment (balanced after per-op HW measurement; v2 was
DVE-bound at 82% busy):
  Scalar : exp(k+u) -> eku ; d*ev / d^2*X1ev scale-multiplies
           (activation Copy with per-partition scale column — lives in
           the same act table as Exp, so zero table reloads) ; num
           PSUM->SBUF bf16 drain.
  GpSimd : ekv = eku*v (Pool is SBUF-only; TT mult is its one fast op).
  DVE    : combine adds X1 = (d*ev) + od, X2 = (d^2*X1ev) + X1od
           (bf16 2x tensor_tensor), the two per-block scans,
           reciprocal_approx_fast (PSUM fp32 -> bf16), and a share of
           the final wkv = num*rden multiplies.
  PE     : phase downsweep — per tensor 12 matmuls of [P,P]@[P,512]
           (s'-terms diag(reu*d^j), P_c terms, identity-z) accumulated
           into PSUM num/den chunks.
The T recurrence is the same phase decomposition as v2: 4 interleaved
phases pair-combined twice, ONE scan of length 512 with decay d^4, and
a TensorE down-sweep reconstructing all phase planes of A_{t-1}.
"""

import numpy as np
from contextlib import ExitStack

import concourse.bass as bass
import concourse.tile as tile
from concourse import mybir, bacc
from concourse.bass_utils import run_bass_kernel_spmd
from concourse.masks import make_identity
from concourse.dve_ops import RECIP_APPROX_FAST_CONSTS, RECIPROCAL_APPROX_FAST

import os

B, T, H = 8, 2048, 768
P = 128
NHB = H // P    # 6 h-blocks
NPH = 4
S = T // NPH    # 512
L = 2
NPOW = L + 1

F32 = mybir.dt.float32
BF16 = mybir.dt.bfloat16

# tuning knobs
POOL_MUL = int(os.environ.get("WKV_POOL_MUL", "6"))    # of 12 chunk-muls on Pool
EKV_DVE = int(os.environ.get("WKV_EKV_DVE", "0"))      # blocks with ekv on DVE
RECIP_F32 = os.environ.get("WKV_RECIP_F32", "0") == "1"
X2F32 = os.environ.get("WKV_X2F32", "0") == "1"
SCALE_SC = os.environ.get("WKV_SCALE_SC", "1") == "1"  # l1/l2 scale on Scalar

_cache = {}


def _build(reps=1, hw_loop=False, nph=NPH, **unused):
    assert nph == NPH
    nc = bacc.Bacc()
    k_in = nc.dram_tensor("k", [H, T], BF16, kind="ExternalInput")
    v_in = nc.dram_tensor("v", [H, T], BF16, kind="ExternalInput")
    dp_in = nc.dram_tensor("dp", [NPOW, H], F32, kind="ExternalInput")
    u_in = nc.dram_tensor("u", [H], F32, kind="ExternalInput")
    reu_in = nc.dram_tensor("reu", [H], F32, kind="ExternalInput")
    rud_in = nc.dram_tensor("rud", [NPH, H], F32, kind="ExternalInput")
    o = nc.dram_tensor("o", [H, T], BF16, kind="ExternalOutput")

    mult, add = mybir.AluOpType.mult, mybir.AluOpType.add
    Copy = mybir.ActivationFunctionType.Copy
    Exp = mybir.ActivationFunctionType.Exp

    with tile.TileContext(nc) as tc, ExitStack() as ctx:
        consts = ctx.enter_context(tc.tile_pool(name="consts", bufs=1))
        work = ctx.enter_context(tc.tile_pool(
            name="work", bufs=int(os.environ.get("WKV_BUFS", "3"))))
        psum = ctx.enter_context(tc.tile_pool(name="psum", bufs=2, space="PSUM"))

        dp_cols = consts.tile([P, NPOW * NHB], F32)
        u_cols = consts.tile([P, NHB], F32)
        nc.sync.dma_start(
            out=dp_cols, in_=dp_in.rearrange("n (f p) -> p (n f)", p=P))
        nc.sync.dma_start(out=u_cols, in_=u_in.rearrange("(f p) -> p f", p=P))

        ident16 = consts.tile([P, P], BF16)
        make_identity(nc, ident16[:])
        rud_cols = consts.tile([P, NPH * NHB], F32)
        nc.sync.dma_start(
            out=rud_cols, in_=rud_in.rearrange("n (f p) -> p (n f)", p=P))
        diag_rud = consts.tile([P, NPH * NHB * P], BF16)
        for _j in range(NPH):
            for _hb in range(NHB):
                _o = (_j * NHB + _hb) * P
                nc.vector.tensor_scalar_mul(
                    out=diag_rud[:, _o:_o + P], in0=ident16,
                    scalar1=rud_cols[:, _j * NHB + _hb:_j * NHB + _hb + 1])

        def dgj(j, hb):
            _o = (j * NHB + hb) * P
            return diag_rud[:, _o:_o + P]

        def dpcol(l, hb):
            return dp_cols[:, l * NHB + hb:l * NHB + hb + 1]

        mul_sched = [(i % 12) < POOL_MUL for i in [(j * 7) % 12 for j in range(12)]]

        import contextlib
        loop_ctx = tc.For_i(0, reps) if hw_loop else contextlib.nullcontext()

        def stage1(hb):
            ucol = u_cols[:, hb:hb + 1]
            hs = slice(hb * P, (hb + 1) * P)
            kb = work.tile([P, T], BF16, tag="kb")
            nc.sync.dma_start(out=kb, in_=k_in[hs, :])
            vb = work.tile([P, T], BF16, tag="vb")
            nc.sync.dma_start(out=vb, in_=v_in[hs, :])
            eku = work.tile([P, T], BF16, tag="eku")
            nc.scalar.activation(out=eku, in_=kb, func=Exp, bias=ucol)
            ekv = work.tile([P, T], BF16, tag="ekv")
            if hb < EKV_DVE:
                nc.vector.tensor_mul(out=ekv, in0=eku, in1=vb)
            else:
                nc.gpsimd.tensor_mul(out=ekv, in0=eku, in1=vb)
            return {"eku": eku, "ekv": ekv}

        def bundle(z, pfx, hb):
            x2dt = F32 if X2F32 else BF16
            pair = z[:, 0:T].rearrange("p (a s) -> p a s", s=2 * S)
            ev = pair[:, :, 0:S]
            od = pair[:, :, S:2 * S]
            X1 = work.tile([P, 2 * S], BF16, tag=f"{pfx}X1")
            X1v = X1.rearrange("p (a s) -> p a s", s=S)
            if SCALE_SC:
                T1 = work.tile([P, 2 * S], BF16, tag=f"{pfx}T1")
                T1v = T1.rearrange("p (a s) -> p a s", s=S)
                nc.scalar.activation(out=T1v, in_=ev, func=Copy,
                                     scale=dpcol(0, hb))
                nc.vector.tensor_tensor(out=X1v, in0=T1v, in1=od, op=add)
            else:
                nc.vector.scalar_tensor_tensor(
                    out=X1v, in0=ev, scalar=dpcol(0, hb), in1=od,
                    op0=mult, op1=add)
            X2 = work.tile([P, S], x2dt, tag=f"{pfx}X2")
            if SCALE_SC:
                T2 = work.tile([P, S], BF16, tag=f"{pfx}T2")
                nc.scalar.activation(out=T2, in_=X1[:, 0:S], func=Copy,
                                     scale=dpcol(1, hb))
                nc.vector.tensor_tensor(out=X2, in0=T2,
                                        in1=X1[:, S:2 * S], op=add)
            else:
                nc.vector.scalar_tensor_tensor(
                    out=X2, in0=X1[:, 0:S], scalar=dpcol(1, hb),
                    in1=X1[:, S:2 * S], op0=mult, op1=add)
            s = work.tile([P, S + 1], BF16, tag=f"{pfx}s")
            nc.gpsimd.memset(s[:, 0:1], 0.0)
            nc.vector.tensor_tensor_scan(
                out=s[:, 1:S + 1],
                data0=dpcol(2, hb).broadcast_to([P, S]),
                data1=X2, initial=0.0, op0=mult, op1=add)
            return {"sp": s[:, 0:S], "X1": X1, "z": z}

        def stage2(hb, st1):
            # B-side first: it does not depend on the Pool ekv multiply
            bB = bundle(st1["eku"], "b", hb)
            bA = bundle(st1["ekv"], "a", hb)
            return {"a": bA, "b": bB}

        def stage3(hb, st2):
            hs = slice(hb * P, (hb + 1) * P)

            def terms_for(spx, X1x, zx, c):
                z0 = zx[:, 0:S]
                z2 = zx[:, 2 * S:3 * S]
                x10 = X1x[:, 0:S]
                tl = [
                    [(dgj(0, hb), spx)],
                    [(dgj(1, hb), spx), (dgj(0, hb), z0)],
                    [(dgj(2, hb), spx), (dgj(0, hb), x10)],
                    [(dgj(3, hb), spx), (dgj(1, hb), x10),
                     (dgj(0, hb), z2)],
                ][c]
                return tl + [(ident16, zx[:, c * S:(c + 1) * S])]

            for ch in range(2):
                num_h = psum.tile([P, 2 * S], F32, tag="numh")
                den_h = psum.tile([P, 2 * S], F32, tag="denh")
                for (acc, bx) in ((num_h, st2["a"]), (den_h, st2["b"])):
                    for cc in range(2):
                        c = ch * 2 + cc
                        tl = terms_for(bx["sp"], bx["X1"], bx["z"], c)
                        for ti, (dgm, mv) in enumerate(tl):
                            nc.tensor.matmul(
                                out=acc[:, cc * S:(cc + 1) * S],
                                lhsT=dgm, rhs=mv, start=(ti == 0),
                                stop=(ti == len(tl) - 1))
                numb = work.tile([P, 2 * S], BF16, tag="numb")
                nc.scalar.activation(out=numb, in_=num_h[:], func=Copy)
                rdt = F32 if RECIP_F32 else BF16
                rdenb = work.tile([P, 2 * S], rdt, tag="rdenb")
                c = RECIP_APPROX_FAST_CONSTS
                nc.vector._custom_dve(
                    RECIPROCAL_APPROX_FAST, out=rdenb[:], in0=den_h[:],
                    s0=c["s0"], s1=c["s1"], imm2=c["imm2"])
                wkv_h = work.tile([P, 2 * S], BF16, tag="wkvh")
                if mul_sched[hb * 2 + ch]:
                    nc.gpsimd.tensor_mul(out=wkv_h, in0=numb, in1=rdenb)
                else:
                    nc.vector.tensor_mul(out=wkv_h, in0=numb, in1=rdenb)
                nc.sync.dma_start(
                    out=o[hs, ch * 2 * S:(ch + 1) * 2 * S], in_=wkv_h)

        with loop_ctx:
          for rep in range(1 if hw_loop else reps):
            s1, s2 = {}, {}
            for i in range(NHB + 2):
                if i < NHB:
                    s1[i] = stage1(i)
                if 1 <= i < NHB + 1:
                    s2[i - 1] = stage2(i - 1, s1[i - 1])
                if i >= 2:
                    stage3(i - 2, s2[i - 2])

    nc.finalize()
    return nc


def prep_host_inputs(key, value, time_decay, time_first, nph=NPH):
    """Host-side prep: [B,T,H] f32 -> per-core [H,T] bf16 phase-major."""
    bf16 = mybir.dt.np(BF16)

    def to_planes(x):
        xt = np.ascontiguousarray(x.T)                  # [H, T]
        xp = xt.reshape(H, S, nph).transpose(0, 2, 1)   # [H, nph, S]
        return np.ascontiguousarray(xp.reshape(H, T)).astype(bf16)

    td64 = np.asarray(time_decay, np.float64)
    u64 = np.asarray(time_first, np.float64)
    d = np.exp(-np.exp(td64))
    dp = np.stack([(d ** (1 << l)) for l in range(L + 1)], axis=0)
    dp = dp.astype(np.float32)
    u = u64.astype(np.float32)
    reu64 = np.exp(-u64)
    reu = reu64.astype(np.float32)
    base = {
        "dp": dp, "u": u, "reu": reu,
        "rud": np.stack([reu64 * (d ** j) for j in range(nph)],
                        axis=0).astype(np.float32),
    }
    return [
        {"k": to_planes(key[b]), "v": to_planes(value[b]), **base}
        for b in range(B)
    ]


def unprep_host_output(o_planes, nph=NPH):
    """[H, T] bf16 phase-major -> [T, H] f32."""
    x = o_planes.astype(np.float32).reshape(H, nph, S)
    xt = x.transpose(0, 2, 1).reshape(H, T)  # [H, T] time-major
    return np.ascontiguousarray(xt.T)


def kernel(key, value, time_decay, time_first):
    key = np.ascontiguousarray(key, dtype=np.float32)
    value = np.ascontiguousarray(value, dtype=np.float32)
    in_maps = prep_host_inputs(key, value, time_decay, time_first)

    if "nc" not in _cache:
        _cache["nc"] = _build(reps=1)
    nc = _cache["nc"]

    res = run_bass_kernel_spmd(nc, in_maps, core_ids=list(range(B)))
    out = np.stack([unprep_host_output(r["o"]) for r in res.results], axis=0)
    return np.ascontiguousarray(out)


if __name__ == "__main__":
    rng = np.random.default_rng(0)
    ktest = rng.standard_normal((B, T, H), dtype=np.float32)
    vtest = rng.standard_normal((B, T, H), dtype=np.float32)
    td = rng.standard_normal(H).astype(np.float32)
    tf = rng.standard_normal(H).astype(np.float32)
    out = kernel(ktest, vtest, td, tf)
    print("out", out.shape, out.dtype, np.abs(out).max())
